# revision 11
# baseline (speedup 1.0000x reference)
"""GroupSort (k=4) Trainium2 Bass kernel.

x: (16384, 4096) f32. Sort each contiguous group of 4 along the last dim.
Sharding: batch-parallel across 8 NeuronCores (2048 rows/core), no comms.

Per core: the 2048x4096 shard is 8 tiles of [128 partitions, 8192 free].
Each partition row holds 2048 groups of 4 contiguous floats; a
5-comparator sorting network (10 min/max tensor_tensor ops on stride-4
views) sorts all groups of a tile. Raw Bass program (no Tile framework —
its semaphore pass emits multi-wait DMA instructions, and the DIRECT2D
DMA ISA struct only has one wait slot; walrus also rejects TensorTensor
on the Pool engine in this toolchain, so all compute is on DVE):

  SP ring:  loads  (HWDGE), double-buffered input
  ACT ring: stores (HWDGE), double-buffered output
  DVE:      5-comparator sorting network, 3 drain-separated blocks

DVE increments its sem at the drain after op 4 (input tile consumed —
unblocks the load 2 tiles ahead) and after op 10 (output tile complete —
unblocks the store). DVE compute is ~183us/core vs the ~187us/core HBM
roofline (64 MiB at ~358 GB/s): memory-bound with near-full overlap.
"""

import numpy as np

B, D, K = 16384, 4096, 4
NCORES = 8
RPC = B // NCORES  # rows per core
N = RPC * D  # flat elements per core
P = 128  # SBUF partitions
F = 8192  # free-dim elements per tile
G = F // K  # groups per partition per tile
NTILES = N // (P * F)  # 8
NBUF = 2

_cache = {}


def _build():
    import concourse.bass as bass
    import concourse.mybir as mybir

    fp32 = mybir.dt.float32
    mn = mybir.AluOpType.min
    mx = mybir.AluOpType.max

    nc = bass.Bass()
    x = nc.dram_tensor("x", [N], fp32, kind="ExternalInput")
    y = nc.dram_tensor("y", [N], fp32, kind="ExternalOutput")
    x_t = x[:].rearrange("(n p f) -> n p f", p=P, f=F)
    y_t = y[:].rearrange("(n p f) -> n p f", p=P, f=F)

    with (
        nc.sbuf_tensor([P, NBUF * F], fp32) as tin,
        nc.sbuf_tensor([P, NBUF * F], fp32) as tout,
        nc.sbuf_tensor([P, 6 * G], fp32) as tmp,
        nc.semaphore("dma_in") as dma_in,
        nc.semaphore("dma_out") as dma_out,
        nc.semaphore("ve") as ve,
        nc.Block() as block,
    ):

        @block.sync
        def _(sync):
            for i in range(NTILES):
                if i > 0:
                    # order completions (also satisfies the sim's sem rule)
                    sync.wait_ge(dma_in, 16 * i)
                if i >= NBUF:
                    # in-slot reuse: readers (ops 1-4 of tile i-NBUF) done
                    sync.wait_ge(ve, 2 * (i - NBUF) + 1)
                sync.dma_start(
                    tin[:, i % NBUF * F : (i % NBUF + 1) * F], x_t[i]
                ).then_inc(dma_in, 16)

        @block.scalar
        def _(scalar):
            for i in range(NTILES):
                if i > 0:
                    scalar.wait_ge(dma_out, 16 * i)
                scalar.wait_ge(ve, 2 * i + 2)
                scalar.dma_start(
                    y_t[i], tout[:, i % NBUF * F : (i % NBUF + 1) * F]
                ).then_inc(dma_out, 16)

        @block.vector
        def _(vector):
            for i in range(NTILES):
                s = i % NBUF
                vi = tin[:, s * F : (s + 1) * F].rearrange(
                    "p (g k) -> p g k", k=K
                )
                vo = tout[:, s * F : (s + 1) * F].rearrange(
                    "p (g k) -> p g k", k=K
                )
                a, b, c, d = (vi[:, :, j] for j in range(K))
                l0, l1, l2, l3 = (vo[:, :, j] for j in range(K))
                lo01, hi01, lo23, hi23, m1, m2 = (
                    tmp[:, j * G : (j + 1) * G] for j in range(6)
                )

                vector.wait_ge(dma_in, 16 * (i + 1))
                if i >= NBUF:
                    # out-slot reuse: store of tile i-NBUF has drained
                    vector.wait_ge(dma_out, 16 * (i - NBUF + 1))
                # drain-separated blocks: no RAW/WAR within a block (the
                # DVE pipeline commits writes only at a DRAIN)
                vector.tensor_tensor(lo01, a, b, mn)
                vector.tensor_tensor(hi01, a, b, mx)
                vector.tensor_tensor(lo23, c, d, mn)
                vector.tensor_tensor(hi23, c, d, mx)
                # the inc also tells the SP ring the input slot is free
                vector.drain().then_inc(ve, 1)
                vector.tensor_tensor(l0, lo01, lo23, mn)
                vector.tensor_tensor(m1, lo01, lo23, mx)
                vector.tensor_tensor(l3, hi01, hi23, mx)
                vector.tensor_tensor(m2, hi01, hi23, mn)
                vector.drain()
                vector.tensor_tensor(l1, m1, m2, mn)
                vector.tensor_tensor(l2, m1, m2, mx)
                # commit before the ACT ring stores this tile
                vector.drain().then_inc(ve, 1)

    return nc


def _run(x_np, trace=False, trace_kwargs=None):
    from concourse.bass_utils import run_bass_kernel_spmd

    if "nc" not in _cache:
        _cache["nc"] = _build()
    nc = _cache["nc"]

    shards = np.split(np.ascontiguousarray(x_np, dtype=np.float32), NCORES, axis=0)
    in_maps = [{"x": s.reshape(-1)} for s in shards]
    res = run_bass_kernel_spmd(
        nc,
        in_maps,
        list(range(NCORES)),
        trace=trace,
        **(trace_kwargs or {}),
    )
    out = np.concatenate([r["y"].reshape(RPC, D) for r in res.results], axis=0)
    return out, res


def kernel(x, k):
    assert int(k) == K, f"kernel hardcodes k={K}, got {k}"
    out, _ = _run(np.asarray(x))
    return out


# revision 12
# speedup vs baseline: 1.3604x; 1.3604x over previous
"""GroupSort (k=4) Trainium2 Bass kernel.

x: (16384, 4096) f32. Sort each contiguous group of 4 along the last dim.
Sharding: batch-parallel across 8 NeuronCores (2048 rows/core), no comms.

Per core: the 2048x4096 shard is 16 tiles of [128 partitions, 4096 free].
A 5-comparator sorting network sorts every contiguous group of 4. DVE ops
with any stride-4 operand run at ~0.59 elem/cycle (measured), so the
network is restructured: pair stages read stride-2 even/odd views and
write contiguous temps, and the four unavoidable stride-4 interleave
writes into the output tile are done by the otherwise-idle Scalar engine
as copies. Raw Bass program (Tile's semaphore pass emits multi-wait DMA
instructions, which the single-wait DIRECT2D ISA struct rejects; walrus
also rejects TensorTensor on Pool in this toolchain):

  SP ring:  loads (HWDGE), double-buffered input
  ACT ring: 4 interleave copies per tile + stores (HWDGE)
  DVE:      8 min/max ops per tile into contiguous temps

Roofline: 64 MiB HBM traffic/core at ~358 GB/s = ~187 us.
"""

import numpy as np

B, D, K = 16384, 4096, 4
NCORES = 8
RPC = B // NCORES  # rows per core
N = RPC * D  # flat elements per core
P = 128  # SBUF partitions
F = 4096  # free-dim elements per tile
G = F // K  # groups per partition per tile
G2 = F // 2
NTILES = N // (P * F)  # 16
NBUF = 2

_cache = {}


def _build():
    import concourse.bass as bass
    import concourse.mybir as mybir

    fp32 = mybir.dt.float32
    mn = mybir.AluOpType.min
    mx = mybir.AluOpType.max

    nc = bass.Bass()
    x = nc.dram_tensor("x", [N], fp32, kind="ExternalInput")
    y = nc.dram_tensor("y", [N], fp32, kind="ExternalOutput")
    x_t = x[:].rearrange("(n p f) -> n p f", p=P, f=F)
    y_t = y[:].rearrange("(n p f) -> n p f", p=P, f=F)

    with (
        nc.sbuf_tensor([P, NBUF * F], fp32) as tin,
        nc.sbuf_tensor([P, NBUF * F], fp32) as tout,
        nc.sbuf_tensor([P, G2], fp32) as lo_pair,  # [lo01 lo23 ...] DVE-only
        nc.sbuf_tensor([P, G2], fp32) as hi_pair,  # [hi01 hi23 ...] DVE-only
        nc.sbuf_tensor([P, G], fp32) as m1,  # DVE-only
        nc.sbuf_tensor([P, G], fp32) as m2,  # DVE-only
        nc.sbuf_tensor([P, NBUF * 4 * G], fp32) as lanes,  # DVE->ACT handoff
        nc.semaphore("dma_in") as dma_in,
        nc.semaphore("dma_out") as dma_out,
        nc.semaphore("ve") as ve,
        nc.semaphore("ac") as ac,
        nc.Block() as block,
    ):

        @block.sync
        def _(sync):
            for i in range(NTILES):
                if i > 0:
                    # order completions (also satisfies the sim's sem rule)
                    sync.wait_ge(dma_in, 16 * i)
                if i >= NBUF:
                    # in-slot reuse: stage-1 of tile i-NBUF consumed it
                    sync.wait_ge(ve, 2 * (i - NBUF) + 1)
                sync.dma_start(
                    tin[:, i % NBUF * F : (i % NBUF + 1) * F], x_t[i]
                ).then_inc(dma_in, 16)

        @block.vector
        def _(vector):
            for i in range(NTILES):
                s = i % NBUF
                vi = tin[:, s * F : (s + 1) * F].rearrange(
                    "p (g k) -> p g k", k=2
                )
                ev, od = vi[:, :, 0], vi[:, :, 1]  # stride-2 views
                vlo = lo_pair[:].rearrange("p (g k) -> p g k", k=2)
                vhi = hi_pair[:].rearrange("p (g k) -> p g k", k=2)
                ln = [
                    lanes[:, (4 * s + j) * G : (4 * s + j + 1) * G]
                    for j in range(4)
                ]

                vector.wait_ge(dma_in, 16 * (i + 1))
                # stage 1: pairwise min/max, stride-2 reads, unit writes
                vector.tensor_tensor(lo_pair[:], ev, od, mn)
                vector.tensor_tensor(hi_pair[:], ev, od, mx)
                # inc: tells the SP ring the input slot is free
                vector.drain().then_inc(ve, 1)
                if i >= NBUF:
                    # handoff-slot reuse: ACT copies of tile i-NBUF done
                    vector.wait_ge(ac, i - NBUF + 1)
                # stage 2: stride-2 reads of the pair arrays, unit writes
                vector.tensor_tensor(ln[0], vlo[:, :, 0], vlo[:, :, 1], mn)
                vector.tensor_tensor(m1[:], vlo[:, :, 0], vlo[:, :, 1], mx)
                vector.tensor_tensor(ln[3], vhi[:, :, 0], vhi[:, :, 1], mx)
                vector.tensor_tensor(m2[:], vhi[:, :, 0], vhi[:, :, 1], mn)
                vector.drain()
                # stage 3: fully unit
                vector.tensor_tensor(ln[1], m1[:], m2[:], mn)
                vector.tensor_tensor(ln[2], m1[:], m2[:], mx)
                # commit before the ACT ring interleaves this tile
                vector.drain().then_inc(ve, 1)

        @block.scalar
        def _(scalar):
            for i in range(NTILES):
                s = i % NBUF
                vo = tout[:, s * F : (s + 1) * F].rearrange(
                    "p (g k) -> p g k", k=K
                )
                ln = [
                    lanes[:, (4 * s + j) * G : (4 * s + j + 1) * G]
                    for j in range(4)
                ]
                scalar.wait_ge(ve, 2 * i + 2)
                if i >= NBUF:
                    # out-slot reuse: store of tile i-NBUF has drained
                    scalar.wait_ge(dma_out, 16 * (i - NBUF + 1))
                for j in range(4):
                    scalar.copy(vo[:, :, j], ln[j])
                # commit copies, free the handoff slot for DVE
                scalar.drain().then_inc(ac, 1)
                if i > 0:
                    scalar.wait_ge(dma_out, 16 * i)
                scalar.dma_start(
                    y_t[i], tout[:, s * F : (s + 1) * F]
                ).then_inc(dma_out, 16)

    return nc


def _run(x_np, trace=False, trace_kwargs=None):
    from concourse.bass_utils import run_bass_kernel_spmd

    if "nc" not in _cache:
        _cache["nc"] = _build()
    nc = _cache["nc"]

    shards = np.split(np.ascontiguousarray(x_np, dtype=np.float32), NCORES, axis=0)
    in_maps = [{"x": s.reshape(-1)} for s in shards]
    res = run_bass_kernel_spmd(
        nc,
        in_maps,
        list(range(NCORES)),
        trace=trace,
        **(trace_kwargs or {}),
    )
    out = np.concatenate([r["y"].reshape(RPC, D) for r in res.results], axis=0)
    return out, res


def kernel(x, k):
    assert int(k) == K, f"kernel hardcodes k={K}, got {k}"
    out, _ = _run(np.asarray(x))
    return out


# revision 13
# speedup vs baseline: 1.3734x; 1.0096x over previous
"""GroupSort (k=4) Trainium2 Bass kernel.

x: (16384, 4096) f32. Sort each contiguous group of 4 along the last dim.
Sharding: batch-parallel across 8 NeuronCores (2048 rows/core), no comms.

Per core: the 2048x4096 shard is 16 tiles of [128 partitions, 4096 free].
A 5-comparator sorting network sorts every contiguous group of 4. DVE ops
with any stride-4 operand run at ~0.59 elem/cycle (measured), so the
network is restructured: pair stages read stride-2 even/odd views and
write contiguous temps, and the four unavoidable stride-4 interleave
writes into the output tile are done by the otherwise-idle Scalar engine
as copies. Raw Bass program (Tile's semaphore pass emits multi-wait DMA
instructions, which the single-wait DIRECT2D ISA struct rejects; walrus
also rejects TensorTensor on Pool in this toolchain):

  SP ring:  loads (HWDGE), double-buffered input
  ACT ring: 4 interleave copies per tile + stores (HWDGE)
  DVE:      8 min/max ops per tile into contiguous temps

Roofline: 64 MiB HBM traffic/core at ~358 GB/s = ~187 us.
"""

import numpy as np

B, D, K = 16384, 4096, 4
NCORES = 8
RPC = B // NCORES  # rows per core
N = RPC * D  # flat elements per core
P = 128  # SBUF partitions
F = 4096  # free-dim elements per tile
G = F // K  # groups per partition per tile
G2 = F // 2
NTILES = N // (P * F)  # 16
NBUF = 3

_cache = {}


def _build():
    import concourse.bass as bass
    import concourse.mybir as mybir

    fp32 = mybir.dt.float32
    mn = mybir.AluOpType.min
    mx = mybir.AluOpType.max

    nc = bass.Bass()
    x = nc.dram_tensor("x", [N], fp32, kind="ExternalInput")
    y = nc.dram_tensor("y", [N], fp32, kind="ExternalOutput")
    x_t = x[:].rearrange("(n p f) -> n p f", p=P, f=F)
    y_t = y[:].rearrange("(n p f) -> n p f", p=P, f=F)

    with (
        nc.sbuf_tensor([P, NBUF * F], fp32) as tin,
        nc.sbuf_tensor([P, NBUF * F], fp32) as tout,
        nc.sbuf_tensor([P, G2], fp32) as lo_pair,  # [lo01 lo23 ...] DVE-only
        nc.sbuf_tensor([P, G2], fp32) as hi_pair,  # [hi01 hi23 ...] DVE-only
        nc.sbuf_tensor([P, G], fp32) as m1,  # DVE-only
        nc.sbuf_tensor([P, G], fp32) as m2,  # DVE-only
        nc.sbuf_tensor([P, NBUF * 4 * G], fp32) as lanes,  # DVE->ACT handoff
        nc.semaphore("dma_in") as dma_in,
        nc.semaphore("dma_out") as dma_out,
        nc.semaphore("ve") as ve,
        nc.semaphore("ac") as ac,
        nc.Block() as block,
    ):

        @block.sync
        def _(sync):
            for i in range(NTILES):
                if i > 0:
                    # order completions (also satisfies the sim's sem rule)
                    sync.wait_ge(dma_in, 16 * i)
                if i >= NBUF:
                    # in-slot reuse: stage-1 of tile i-NBUF consumed it
                    sync.wait_ge(ve, 2 * (i - NBUF) + 1)
                sync.dma_start(
                    tin[:, i % NBUF * F : (i % NBUF + 1) * F], x_t[i]
                ).then_inc(dma_in, 16)

        @block.vector
        def _(vector):
            for i in range(NTILES):
                s = i % NBUF
                vi = tin[:, s * F : (s + 1) * F].rearrange(
                    "p (g k) -> p g k", k=2
                )
                ev, od = vi[:, :, 0], vi[:, :, 1]  # stride-2 views
                vlo = lo_pair[:].rearrange("p (g k) -> p g k", k=2)
                vhi = hi_pair[:].rearrange("p (g k) -> p g k", k=2)
                ln = [
                    lanes[:, (4 * s + j) * G : (4 * s + j + 1) * G]
                    for j in range(4)
                ]

                vector.wait_ge(dma_in, 16 * (i + 1))
                # stage 1: pairwise min/max, stride-2 reads, unit writes
                vector.tensor_tensor(lo_pair[:], ev, od, mn)
                vector.tensor_tensor(hi_pair[:], ev, od, mx)
                # inc: tells the SP ring the input slot is free
                vector.drain().then_inc(ve, 1)
                if i >= NBUF:
                    # handoff-slot reuse: ACT copies of tile i-NBUF done
                    vector.wait_ge(ac, i - NBUF + 1)
                # stage 2: stride-2 reads of the pair arrays, unit writes
                vector.tensor_tensor(ln[0], vlo[:, :, 0], vlo[:, :, 1], mn)
                vector.tensor_tensor(m1[:], vlo[:, :, 0], vlo[:, :, 1], mx)
                vector.tensor_tensor(ln[3], vhi[:, :, 0], vhi[:, :, 1], mx)
                vector.tensor_tensor(m2[:], vhi[:, :, 0], vhi[:, :, 1], mn)
                vector.drain()
                # stage 3: fully unit
                vector.tensor_tensor(ln[1], m1[:], m2[:], mn)
                vector.tensor_tensor(ln[2], m1[:], m2[:], mx)
                # commit before the ACT ring interleaves this tile
                vector.drain().then_inc(ve, 1)

        @block.scalar
        def _(scalar):
            for i in range(NTILES):
                s = i % NBUF
                vo = tout[:, s * F : (s + 1) * F].rearrange(
                    "p (g k) -> p g k", k=K
                )
                ln = [
                    lanes[:, (4 * s + j) * G : (4 * s + j + 1) * G]
                    for j in range(4)
                ]
                scalar.wait_ge(ve, 2 * i + 2)
                if i >= NBUF:
                    # out-slot reuse: store of tile i-NBUF has drained
                    scalar.wait_ge(dma_out, 16 * (i - NBUF + 1))
                for j in range(4):
                    scalar.copy(vo[:, :, j], ln[j])
                # commit copies, free the handoff slot for DVE
                scalar.drain().then_inc(ac, 1)
                if i > 0:
                    scalar.wait_ge(dma_out, 16 * i)
                scalar.dma_start(
                    y_t[i], tout[:, s * F : (s + 1) * F]
                ).then_inc(dma_out, 16)

    return nc


def _run(x_np, trace=False, trace_kwargs=None):
    from concourse.bass_utils import run_bass_kernel_spmd

    if "nc" not in _cache:
        _cache["nc"] = _build()
    nc = _cache["nc"]

    shards = np.split(np.ascontiguousarray(x_np, dtype=np.float32), NCORES, axis=0)
    in_maps = [{"x": s.reshape(-1)} for s in shards]
    res = run_bass_kernel_spmd(
        nc,
        in_maps,
        list(range(NCORES)),
        trace=trace,
        **(trace_kwargs or {}),
    )
    out = np.concatenate([r["y"].reshape(RPC, D) for r in res.results], axis=0)
    return out, res


def kernel(x, k):
    assert int(k) == K, f"kernel hardcodes k={K}, got {k}"
    out, _ = _run(np.asarray(x))
    return out
